# revision 8
# baseline (speedup 1.0000x reference)
"""BiLSTM Trainium2 kernel.

Reference semantics (hk.LSTM, haiku):
    gated = [x_t, h_{t-1}] @ W + b          # [B, 4H], gate order i, g, f, o
    f = sigmoid(f_raw + 1)
    c = f * c + sigmoid(i) * tanh(g)
    h = sigmoid(o) * tanh(c)
Forward over t for y[:, :, :H] (weights W1), backward over t for
y[:, :, H:] (weights W2).

Sharding: 8 cores SPMD. Cores 0-3 run the forward direction on batch
rows 8j..8j+7; cores 4-7 run the backward direction on the same batch
quarters with time-reversed input (host flips, so every core computes an
identical "forward" scan). Host re-flips/concats outputs.

Per-core kernel layout (B=8 sequences, T=1024 steps, D=512, H=256):
  - x is host-transposed/padded to x~^T [640, T, B]: rows 0-511 input
    features, row 512 = 1.0 (bias row), 513-639 zero pad. Gate columns
    of W for g are pre-scaled x2 (tanh(g) = 2*sigmoid(2g) - 1, so the
    single ACT table set "sigmoid" covers everything) and the f columns
    of the bias row carry the haiku +1 forget bias.
  - Input projections u_t = x~_t @ W~x computed chunk-wise (CH steps)
    straight into PSUM [128, (m, t, b)] via weights-stationary matmuls.
  - Recurrence: per step, 16 matmuls (2 K-tiles x 8 M-tiles) accumulate
    h_{t-1} @ Wh onto the PSUM gates (start=False), giving gates already
    transposed: partition = gate channel, free = (m, b). One sigmoid ACT
    covers all four gates; g is fixed up with 2s-1; DVE updates c; tanh
    ACT + DVE produce h directly into the y output ring, which doubles
    as the next step's stationary matmul operand.
"""

import os
import sys

if "/opt/trn_rl_repo" not in sys.path:
    sys.path.insert(0, "/opt/trn_rl_repo")
os.environ.setdefault("JAX_COMPILATION_CACHE_DIR", "/tmp/jax_cache")
os.environ.setdefault("JAX_PERSISTENT_CACHE_MIN_COMPILE_TIME_SECS", "10")

import numpy as np

import bass_rust
import concourse.bass as bass
import concourse.mybir as mybir
import concourse.tile as tile
from concourse.vector_clock import ScopedClock
from concourse.bass_utils import run_bass_kernel_spmd

# ----------------------------------------------------------------------------
# Problem constants (hardcoded per contest contract)
B_FULL = 32
T_FULL = 1024
D = 512  # input features
H = 256  # hidden
G = 4 * H  # gate width 1024
N_CORES = 8
B_CORE = 8  # batch rows per core

# Kernel config
DT_STR = "float32"  # compute dtype for x / W / h ("float32"|"float16"|"bfloat16")
CH = 16  # recurrence chunk length (steps per PSUM u-block)

KX = 5  # k-tiles for the padded input projection (640 = 5*128)
KH = 2  # k-tiles for the recurrent matmul (256 = 2*128)
M = 8  # gate m-tiles (1024 = 8*128)


class _TC(tile.TileContext):
    """TileContext whose final drain splits sem waits 1-per-instruction.

    The walrus build in this container rejects >1 sync wait on a CTRL
    (Drain) instruction; stock Tile attaches the whole end-of-kernel
    vector clock to a single drain.
    """

    MAX_DRAIN_WAITS = 1

    def _drain_and_barrier(self, tick_clock, wait_clock):
        drain_inst = self.nc.sync.drain()
        wait_clock.add_sem_waits(
            drain_inst.ins, ScopedClock({None: tick_clock.global_clock})
        )
        si = drain_inst.ins.sync_info
        if si is not None and si.on_wait and len(si.on_wait) > self.MAX_DRAIN_WAITS:
            waits = list(si.on_wait)
            si.on_wait = waits[: self.MAX_DRAIN_WAITS]
            rest = waits[self.MAX_DRAIN_WAITS :]
            for i in range(0, len(rest), self.MAX_DRAIN_WAITS):
                extra = self.nc.sync.drain()
                extra.ins.sync_info = bass_rust.SyncInfo(
                    on_wait=rest[i : i + self.MAX_DRAIN_WAITS], on_update=[]
                )
        self.nc.all_engine_barrier()
        assert self.sems is not None
        popped = self.nc._tile_sem_poison_stack.pop()
        assert popped is self._sem_poison
        self.nc.clear_and_free_semaphores(list(self.sems.allocated().values()))
        self.nc.all_engine_barrier()


def _split_excess_waits(nc, limit=1):
    """Walrus in this container accepts at most `limit` sync waits per
    instruction; move excess waits onto same-engine NoOp carriers placed
    immediately before the over-limit instruction (NX dispatch is in-order,
    so a preceding nop's waits gate the instruction identically)."""
    n_carriers = 0
    for fn in nc.m.functions:
        for bb in fn.blocks:
            out = []
            for inst in bb.instructions:
                si = inst.sync_info
                if si is not None and si.on_wait and len(si.on_wait) > limit:
                    waits = list(si.on_wait)
                    rest, keep = waits[:-limit], waits[-limit:]
                    for i in range(0, len(rest), limit):
                        nop = bass_rust.InstNoOp(
                            name=nc.get_next_instruction_name(), ins=[], outs=[]
                        )
                        nop.engine = inst.engine
                        nop.sync_info = bass_rust.SyncInfo(
                            on_wait=rest[i : i + limit], on_update=[]
                        )
                        nc.register_instruction(nop, overwrite=True)
                        out.append(nop)
                        n_carriers += 1
                    si.on_wait = keep
                out.append(inst)
            bb.instructions = out
    return n_carriers


def build_nc(dt_str=DT_STR, T=T_FULL, ch=CH, b=B_CORE):
    """Build the per-core Bass program (SPMD across all 8 cores)."""
    DT = getattr(mybir.dt, dt_str)
    F32 = mybir.dt.float32
    AF = mybir.ActivationFunctionType
    OP = mybir.AluOpType
    n_ch = T // ch
    assert T % ch == 0

    nc = bass.Bass()
    xt = nc.dram_tensor("xt", [KX * 128, T, b], DT, kind="ExternalInput")
    wx = nc.dram_tensor("wx", [KX * 128, G], DT, kind="ExternalInput")
    wh = nc.dram_tensor("wh", [KH * 128, G], DT, kind="ExternalInput")
    y = nc.dram_tensor("y", [128, T, KH * b], DT, kind="ExternalOutput")

    xt_v = xt.rearrange("(k p) t b -> p k t b", p=128)
    wx_v = wx.rearrange("(k p) (m q) -> p k m q", p=128, q=128)
    wh_v = wh.rearrange("(k p) (m q) -> p k m q", p=128, q=128)

    with _TC(nc) as tc:
        with (
            tc.tile_pool(name="consts", bufs=1) as cpool,
            tc.tile_pool(name="xring", bufs=2) as xpool,
            tc.tile_pool(name="yring", bufs=2) as ypool,
            tc.tile_pool(name="steps", bufs=3) as spool,
            tc.tile_pool(name="uring", bufs=2) as upool,
            tc.tile_pool(name="psum", bufs=2, space="PSUM") as ppool,
            tc.tile_pool(name="rpsum", bufs=3, space="PSUM") as rpool,
        ):
            # Resident weights: [128, (k m) * 128]
            wx_sb = cpool.tile([128, KX * M * 128], DT)
            wh_sb = cpool.tile([128, KH * M * 128], DT)
            nc.sync.dma_start(
                wx_sb[:].rearrange("p (k m q) -> p k m q", k=KX, m=M), wx_v[:]
            )
            nc.sync.dma_start(
                wh_sb[:].rearrange("p (k m q) -> p k m q", k=KH, m=M), wh_v[:]
            )
            wx_t = wx_sb[:].rearrange("p (km q) -> p km q", q=128)
            wh_t = wh_sb[:].rearrange("p (km q) -> p km q", q=128)

            # Persistent state
            h0 = cpool.tile([128, KH * b], DT, tag="h0")
            c_st = cpool.tile([128, KH * b], F32, tag="c")
            nc.vector.memset(h0[:], 0.0)
            nc.vector.memset(c_st[:], 0.0)

            prev_ych = None
            for c_i in range(n_ch):
                t0 = c_i * ch
                # ---- load x chunk, project u into PSUM, copy to SBUF -----
                xch = xpool.tile([128, KX * ch * b], DT, tag="xch")
                nc.sync.dma_start(
                    xch[:].rearrange("p (k t b) -> p k t b", k=KX, t=ch),
                    xt_v[:, :, t0 : t0 + ch, :],
                )
                xch_v = xch[:].rearrange("p (k t b) -> p k (t b)", k=KX, t=ch)

                ups = ppool.tile([128, M * ch * b], F32, tag="upsum")
                ups_m = ups[:].rearrange("p (m t b) -> p m (t b)", m=M, t=ch)
                for m in range(M):
                    for k in range(KX):
                        nc.tensor.matmul(
                            ups_m[:, m, :],
                            wx_t[:, k * M + m, :],
                            xch_v[:, k, :],
                            start=(k == 0),
                            stop=(k == KX - 1),
                        )
                uch = upool.tile([128, M * ch * b], F32, tag="uch")
                nc.scalar.copy(uch[:], ups[:])
                uch_s = uch[:].rearrange("p (m t b) -> p m t b", m=M, t=ch)

                # ---- y ring for this chunk (doubles as h storage) --------
                ych = ypool.tile([128, ch * KH * b], DT, tag="ych")
                ych_v = ych[:].rearrange("p (t k b) -> p t k b", t=ch, k=KH)

                for t in range(ch):
                    # h_{t-1} source
                    if t > 0:
                        hsrc = ych_v[:, t - 1, :, :]
                    elif prev_ych is not None:
                        hsrc = prev_ych[:, ch - 1, :, :]
                    else:
                        hsrc = h0[:].rearrange("p (k b) -> p k b", k=KH)

                    # recurrent matmuls into a fresh PSUM tile
                    rec = rpool.tile([128, M * b], F32, tag="rec")
                    rec_m = rec[:].rearrange("p (m b) -> p m b", m=M)
                    for m in range(M):
                        for k in range(KH):
                            nc.tensor.matmul(
                                rec_m[:, m, :],
                                wh_t[:, k * M + m, :],
                                hsrc[:, k, :],
                                start=(k == 0),
                                stop=(k == KH - 1),
                            )

                    # gates = u + rec, then sigma over all four gate groups
                    G_t = spool.tile([128, M * b], F32, tag="G")
                    nc.vector.tensor_tensor(
                        G_t[:], rec[:], uch_s[:, :, t, :], OP.add
                    )
                    S = spool.tile([128, M * b], F32, tag="S")
                    nc.scalar.activation(S[:], G_t[:], AF.Sigmoid)
                    # g fixup: tanh(g) = 2*sigma(2g) - 1 (2g folded into W)
                    g_sl = S[:, 2 * b : 4 * b]
                    nc.vector.tensor_scalar(g_sl, g_sl, 2.0, 1.0, OP.mult, OP.subtract)

                    i_sl = S[:, 0 : 2 * b]
                    f_sl = S[:, 4 * b : 6 * b]
                    o_sl = S[:, 6 * b : 8 * b]

                    # c = f*c + i*g
                    tmp = spool.tile([128, KH * b], F32, tag="tmp")
                    nc.vector.tensor_tensor(tmp[:], i_sl, g_sl, OP.mult)
                    nc.vector.tensor_tensor(c_st[:], f_sl, c_st[:], OP.mult)
                    nc.vector.tensor_tensor(c_st[:], c_st[:], tmp[:], OP.add)

                    # h = o * tanh(c)  (written into y ring, dtype DT)
                    tc_t = spool.tile([128, KH * b], F32, tag="tanh")
                    nc.scalar.activation(tc_t[:], c_st[:], AF.Tanh)
                    nc.vector.tensor_tensor(
                        ych_v[:, t, :, :], o_sl, tc_t[:], OP.mult
                    )

                # ---- store y chunk --------------------------------------
                nc.sync.dma_start(y[:, t0 : t0 + ch, :], ych[:])
                prev_ych = ych_v

    _split_excess_waits(nc)
    return nc


def _prep_core_inputs(x, W, bvec, dt_np, reverse):
    """Build per-core input dict. x: [b, T, D] fp32 (already batch-sliced)."""
    b, T, _ = x.shape
    if reverse:
        x = x[:, ::-1, :]
    # x~^T [KX*128, T, b]
    xt = np.zeros((KX * 128, T, b), np.float32)
    xt[:D] = x.transpose(2, 1, 0)
    xt[D] = 1.0

    # W~x [KX*128, G]: rows 0..D-1 = W_x, row D = bias, g-cols x2, f-bias +1
    wx = np.zeros((KX * 128, G), np.float32)
    wx[:D] = W[:D]
    beff = bvec.astype(np.float32).copy()
    beff[2 * H : 3 * H] += 1.0  # haiku forget-gate bias (f block)
    wx[D] = beff
    wx[:, H : 2 * H] *= 2.0  # g block pre-scale

    wh = W[D:].astype(np.float32).copy()
    wh[:, H : 2 * H] *= 2.0

    return {
        "xt": xt.astype(dt_np),
        "wx": wx.astype(dt_np),
        "wh": wh.astype(dt_np),
    }


def _decode_y(arr):
    """[128, T, KH*b] device layout -> [H, T, b] (h channel = k*128 + p)."""
    a = np.asarray(arr, np.float32)
    p, T, kb = a.shape
    a = a.reshape(p, T, KH, kb // KH)  # [128, T, k, b]
    return a.transpose(2, 0, 1, 3).reshape(KH * 128, T, kb // KH)


def kernel(x, W1, b1, W2, b2):
    x = np.asarray(x, np.float32)
    W1 = np.asarray(W1, np.float32)
    W2 = np.asarray(W2, np.float32)
    b1 = np.asarray(b1, np.float32)
    b2 = np.asarray(b2, np.float32)

    dt_np = {"float32": np.float32, "float16": np.float16}.get(DT_STR)
    if dt_np is None:
        import ml_dtypes

        dt_np = np.dtype(ml_dtypes.bfloat16)

    nc = build_nc(DT_STR, T_FULL, CH, B_CORE)

    in_maps = []
    for j in range(4):
        xs = x[B_CORE * j : B_CORE * (j + 1)]
        in_maps.append(_prep_core_inputs(xs, W1, b1, dt_np, reverse=False))
    for j in range(4):
        xs = x[B_CORE * j : B_CORE * (j + 1)]
        in_maps.append(_prep_core_inputs(xs, W2, b2, dt_np, reverse=True))

    res = run_bass_kernel_spmd(nc, in_maps, list(range(N_CORES)))

    y = np.empty((B_FULL, T_FULL, 2 * H), np.float32)
    for j in range(4):
        yf = _decode_y(res.results[j]["y"])  # [H, T, b]
        y[B_CORE * j : B_CORE * (j + 1), :, :H] = yf.transpose(2, 1, 0)
        yb = _decode_y(res.results[4 + j]["y"])
        y[B_CORE * j : B_CORE * (j + 1), :, H:] = yb[:, ::-1, :].transpose(2, 1, 0)
    return y


# revision 9
# speedup vs baseline: 54.5371x; 54.5371x over previous
"""BiLSTM Trainium2 kernel.

Reference semantics (hk.LSTM, haiku):
    gated = [x_t, h_{t-1}] @ W + b          # [B, 4H], gate order i, g, f, o
    f = sigmoid(f_raw + 1)
    c = f * c + sigmoid(i) * tanh(g)
    h = sigmoid(o) * tanh(c)
Forward over t for y[:, :, :H] (weights W1), backward over t for
y[:, :, H:] (weights W2).

Sharding: 8 cores SPMD. Cores 0-3 run the forward direction on batch
rows 8j..8j+7; cores 4-7 run the backward direction on the same batch
quarters with time-reversed input (host flips, so every core computes an
identical "forward" scan). Host re-flips/concats outputs.

Per-core kernel layout (B=8 sequences, T=1024 steps, D=512, H=256):
  - x is host-transposed/padded to x~^T [640, T, B]: rows 0-511 input
    features, row 512 = 1.0 (bias row), 513-639 zero pad. Gate columns
    of W for g are pre-scaled x2 (tanh(g) = 2*sigmoid(2g) - 1, so the
    single ACT table set "sigmoid" covers everything) and the f columns
    of the bias row carry the haiku +1 forget bias.
  - Input projections u_t = x~_t @ W~x computed chunk-wise (CH steps)
    straight into PSUM [128, (m, t, b)] via weights-stationary matmuls.
  - Recurrence: per step, 16 matmuls (2 K-tiles x 8 M-tiles) accumulate
    h_{t-1} @ Wh onto the PSUM gates (start=False), giving gates already
    transposed: partition = gate channel, free = (m, b). One sigmoid ACT
    covers all four gates; g is fixed up with 2s-1; DVE updates c; tanh
    ACT + DVE produce h directly into the y output ring, which doubles
    as the next step's stationary matmul operand.
"""

import os
import sys

if "/opt/trn_rl_repo" not in sys.path:
    sys.path.insert(0, "/opt/trn_rl_repo")
os.environ.setdefault("JAX_COMPILATION_CACHE_DIR", "/tmp/jax_cache")
os.environ.setdefault("JAX_PERSISTENT_CACHE_MIN_COMPILE_TIME_SECS", "10")

import numpy as np

import bass_rust
import concourse.bass as bass
import concourse.mybir as mybir
import concourse.tile as tile
from concourse.vector_clock import ScopedClock
from concourse.bass_utils import run_bass_kernel_spmd

# ----------------------------------------------------------------------------
# Problem constants (hardcoded per contest contract)
B_FULL = 32
T_FULL = 1024
D = 512  # input features
H = 256  # hidden
G = 4 * H  # gate width 1024
N_CORES = 8
B_CORE = 8  # batch rows per core

# Kernel config
DT_STR = "float16"  # compute dtype for x / W / h ("float32"|"float16"|"bfloat16")
CH = 16  # recurrence chunk length (steps per PSUM u-block)

KX = 5  # k-tiles for the padded input projection (640 = 5*128)
KH = 2  # k-tiles for the recurrent matmul (256 = 2*128)
M = 8  # gate m-tiles (1024 = 8*128)


class _TC(tile.TileContext):
    """TileContext whose final drain splits sem waits 1-per-instruction.

    The walrus build in this container rejects >1 sync wait on a CTRL
    (Drain) instruction; stock Tile attaches the whole end-of-kernel
    vector clock to a single drain.
    """

    MAX_DRAIN_WAITS = 1

    def _drain_and_barrier(self, tick_clock, wait_clock):
        drain_inst = self.nc.sync.drain()
        wait_clock.add_sem_waits(
            drain_inst.ins, ScopedClock({None: tick_clock.global_clock})
        )
        si = drain_inst.ins.sync_info
        if si is not None and si.on_wait and len(si.on_wait) > self.MAX_DRAIN_WAITS:
            waits = list(si.on_wait)
            si.on_wait = waits[: self.MAX_DRAIN_WAITS]
            rest = waits[self.MAX_DRAIN_WAITS :]
            for i in range(0, len(rest), self.MAX_DRAIN_WAITS):
                extra = self.nc.sync.drain()
                extra.ins.sync_info = bass_rust.SyncInfo(
                    on_wait=rest[i : i + self.MAX_DRAIN_WAITS], on_update=[]
                )
        self.nc.all_engine_barrier()
        assert self.sems is not None
        popped = self.nc._tile_sem_poison_stack.pop()
        assert popped is self._sem_poison
        self.nc.clear_and_free_semaphores(list(self.sems.allocated().values()))
        self.nc.all_engine_barrier()


def _split_excess_waits(nc, limit=1):
    """Walrus in this container accepts at most `limit` sync waits per
    instruction; move excess waits onto same-engine NoOp carriers placed
    immediately before the over-limit instruction (NX dispatch is in-order,
    so a preceding nop's waits gate the instruction identically)."""
    n_carriers = 0
    for fn in nc.m.functions:
        for bb in fn.blocks:
            out = []
            for inst in bb.instructions:
                si = inst.sync_info
                if si is not None and si.on_wait and len(si.on_wait) > limit:
                    waits = list(si.on_wait)
                    rest, keep = waits[:-limit], waits[-limit:]
                    for i in range(0, len(rest), limit):
                        nop = bass_rust.InstNoOp(
                            name=nc.get_next_instruction_name(), ins=[], outs=[]
                        )
                        nop.engine = inst.engine
                        nop.sync_info = bass_rust.SyncInfo(
                            on_wait=rest[i : i + limit], on_update=[]
                        )
                        nc.register_instruction(nop, overwrite=True)
                        out.append(nop)
                        n_carriers += 1
                    si.on_wait = keep
                out.append(inst)
            bb.instructions = out
    return n_carriers


def build_nc(dt_str=DT_STR, T=T_FULL, ch=CH, b=B_CORE):
    """Build the per-core Bass program (SPMD across all 8 cores)."""
    DT = getattr(mybir.dt, dt_str)
    F32 = mybir.dt.float32
    AF = mybir.ActivationFunctionType
    OP = mybir.AluOpType
    n_ch = T // ch
    assert T % ch == 0

    nc = bass.Bass()
    xt = nc.dram_tensor("xt", [KX * 128, T, b], DT, kind="ExternalInput")
    wx = nc.dram_tensor("wx", [KX * 128, G], DT, kind="ExternalInput")
    wh = nc.dram_tensor("wh", [KH * 128, G], DT, kind="ExternalInput")
    y = nc.dram_tensor("y", [128, T, KH * b], DT, kind="ExternalOutput")

    xt_v = xt.rearrange("(k p) t b -> p k t b", p=128)
    wx_v = wx.rearrange("(k p) (m q) -> p k m q", p=128, q=128)
    wh_v = wh.rearrange("(k p) (m q) -> p k m q", p=128, q=128)

    with _TC(nc) as tc:
        with (
            tc.tile_pool(name="consts", bufs=1) as cpool,
            tc.tile_pool(name="xring", bufs=2) as xpool,
            tc.tile_pool(name="yring", bufs=2) as ypool,
            tc.tile_pool(name="steps", bufs=3) as spool,
            tc.tile_pool(name="uring", bufs=2) as upool,
            tc.tile_pool(name="psum", bufs=2, space="PSUM") as ppool,
            tc.tile_pool(name="rpsum", bufs=3, space="PSUM") as rpool,
        ):
            # Resident weights: [128, (k m) * 128]
            wx_sb = cpool.tile([128, KX * M * 128], DT)
            wh_sb = cpool.tile([128, KH * M * 128], DT)
            nc.sync.dma_start(
                wx_sb[:].rearrange("p (k m q) -> p k m q", k=KX, m=M), wx_v[:]
            )
            nc.sync.dma_start(
                wh_sb[:].rearrange("p (k m q) -> p k m q", k=KH, m=M), wh_v[:]
            )
            wx_t = wx_sb[:].rearrange("p (km q) -> p km q", q=128)
            wh_t = wh_sb[:].rearrange("p (km q) -> p km q", q=128)

            # Persistent state
            h0 = cpool.tile([128, KH * b], DT, tag="h0")
            c_st = cpool.tile([128, KH * b], F32, tag="c")
            nc.vector.memset(h0[:], 0.0)
            nc.vector.memset(c_st[:], 0.0)

            prev_ych = None
            for c_i in range(n_ch):
                t0 = c_i * ch
                # ---- load x chunk, project u into PSUM, copy to SBUF -----
                xch = xpool.tile([128, KX * ch * b], DT, tag="xch")
                nc.sync.dma_start(
                    xch[:].rearrange("p (k t b) -> p k t b", k=KX, t=ch),
                    xt_v[:, :, t0 : t0 + ch, :],
                )
                xch_v = xch[:].rearrange("p (k t b) -> p k (t b)", k=KX, t=ch)

                ups = ppool.tile([128, M * ch * b], F32, tag="upsum")
                ups_m = ups[:].rearrange("p (m t b) -> p m (t b)", m=M, t=ch)
                for m in range(M):
                    for k in range(KX):
                        nc.tensor.matmul(
                            ups_m[:, m, :],
                            wx_t[:, k * M + m, :],
                            xch_v[:, k, :],
                            start=(k == 0),
                            stop=(k == KX - 1),
                        )
                uch = upool.tile([128, M * ch * b], F32, tag="uch")
                nc.scalar.copy(uch[:], ups[:])
                uch_s = uch[:].rearrange("p (m t b) -> p m t b", m=M, t=ch)

                # ---- y ring for this chunk (doubles as h storage) --------
                ych = ypool.tile([128, ch * KH * b], DT, tag="ych")
                ych_v = ych[:].rearrange("p (t k b) -> p t k b", t=ch, k=KH)

                for t in range(ch):
                    # h_{t-1} source
                    if t > 0:
                        hsrc = ych_v[:, t - 1, :, :]
                    elif prev_ych is not None:
                        hsrc = prev_ych[:, ch - 1, :, :]
                    else:
                        hsrc = h0[:].rearrange("p (k b) -> p k b", k=KH)

                    # recurrent matmuls into a fresh PSUM tile
                    rec = rpool.tile([128, M * b], F32, tag="rec")
                    rec_m = rec[:].rearrange("p (m b) -> p m b", m=M)
                    for m in range(M):
                        for k in range(KH):
                            nc.tensor.matmul(
                                rec_m[:, m, :],
                                wh_t[:, k * M + m, :],
                                hsrc[:, k, :],
                                start=(k == 0),
                                stop=(k == KH - 1),
                            )

                    # gates = u + rec, then sigma over all four gate groups
                    G_t = spool.tile([128, M * b], F32, tag="G")
                    nc.vector.tensor_tensor(
                        G_t[:], rec[:], uch_s[:, :, t, :], OP.add
                    )
                    S = spool.tile([128, M * b], F32, tag="S")
                    nc.scalar.activation(S[:], G_t[:], AF.Sigmoid)
                    # g fixup: tanh(g) = 2*sigma(2g) - 1 (2g folded into W)
                    g_sl = S[:, 2 * b : 4 * b]
                    nc.vector.tensor_scalar(g_sl, g_sl, 2.0, 1.0, OP.mult, OP.subtract)

                    i_sl = S[:, 0 : 2 * b]
                    f_sl = S[:, 4 * b : 6 * b]
                    o_sl = S[:, 6 * b : 8 * b]

                    # c = f*c + i*g
                    tmp = spool.tile([128, KH * b], F32, tag="tmp")
                    nc.vector.tensor_tensor(tmp[:], i_sl, g_sl, OP.mult)
                    nc.vector.tensor_tensor(c_st[:], f_sl, c_st[:], OP.mult)
                    nc.vector.tensor_tensor(c_st[:], c_st[:], tmp[:], OP.add)

                    # h = o * tanh(c)  (written into y ring, dtype DT)
                    tc_t = spool.tile([128, KH * b], F32, tag="tanh")
                    nc.scalar.activation(tc_t[:], c_st[:], AF.Tanh)
                    nc.vector.tensor_tensor(
                        ych_v[:, t, :, :], o_sl, tc_t[:], OP.mult
                    )

                # ---- store y chunk --------------------------------------
                nc.sync.dma_start(y[:, t0 : t0 + ch, :], ych[:])
                prev_ych = ych_v

    _split_excess_waits(nc)
    return nc


def _prep_core_inputs(x, W, bvec, dt_np, reverse):
    """Build per-core input dict. x: [b, T, D] fp32 (already batch-sliced)."""
    b, T, _ = x.shape
    if reverse:
        x = x[:, ::-1, :]
    # x~^T [KX*128, T, b]
    xt = np.zeros((KX * 128, T, b), np.float32)
    xt[:D] = x.transpose(2, 1, 0)
    xt[D] = 1.0

    # W~x [KX*128, G]: rows 0..D-1 = W_x, row D = bias, g-cols x2, f-bias +1
    wx = np.zeros((KX * 128, G), np.float32)
    wx[:D] = W[:D]
    beff = bvec.astype(np.float32).copy()
    beff[2 * H : 3 * H] += 1.0  # haiku forget-gate bias (f block)
    wx[D] = beff
    wx[:, H : 2 * H] *= 2.0  # g block pre-scale

    wh = W[D:].astype(np.float32).copy()
    wh[:, H : 2 * H] *= 2.0

    return {
        "xt": xt.astype(dt_np),
        "wx": wx.astype(dt_np),
        "wh": wh.astype(dt_np),
    }


def _decode_y(arr):
    """[128, T, KH*b] device layout -> [H, T, b] (h channel = k*128 + p)."""
    a = np.asarray(arr, np.float32)
    p, T, kb = a.shape
    a = a.reshape(p, T, KH, kb // KH)  # [128, T, k, b]
    return a.transpose(2, 0, 1, 3).reshape(KH * 128, T, kb // KH)


def kernel(x, W1, b1, W2, b2):
    x = np.asarray(x, np.float32)
    W1 = np.asarray(W1, np.float32)
    W2 = np.asarray(W2, np.float32)
    b1 = np.asarray(b1, np.float32)
    b2 = np.asarray(b2, np.float32)

    dt_np = {"float32": np.float32, "float16": np.float16}.get(DT_STR)
    if dt_np is None:
        import ml_dtypes

        dt_np = np.dtype(ml_dtypes.bfloat16)

    nc = build_nc(DT_STR, T_FULL, CH, B_CORE)

    in_maps = []
    for j in range(4):
        xs = x[B_CORE * j : B_CORE * (j + 1)]
        in_maps.append(_prep_core_inputs(xs, W1, b1, dt_np, reverse=False))
    for j in range(4):
        xs = x[B_CORE * j : B_CORE * (j + 1)]
        in_maps.append(_prep_core_inputs(xs, W2, b2, dt_np, reverse=True))

    res = run_bass_kernel_spmd(nc, in_maps, list(range(N_CORES)))

    y = np.empty((B_FULL, T_FULL, 2 * H), np.float32)
    for j in range(4):
        yf = _decode_y(res.results[j]["y"])  # [H, T, b]
        y[B_CORE * j : B_CORE * (j + 1), :, :H] = yf.transpose(2, 1, 0)
        yb = _decode_y(res.results[4 + j]["y"])
        y[B_CORE * j : B_CORE * (j + 1), :, H:] = yb[:, ::-1, :].transpose(2, 1, 0)
    return y
